# revision 10
# baseline (speedup 1.0000x reference)
"""Distributed TRN2 Bass kernel v4 for causal multi-head attention
(B=2, L=2048, D=1024, H=16, HD=64) on 8 NeuronCores.

Sharding: tensor-parallel over heads (2 heads/core, full sequence), with the
output projection re-sharded by sequence via one AllToAll per batch.  Core r
ends up with all 16 heads for seq rows [r*256,(r+1)*256) of each batch and
computes that slice of the output projection.

v4 on top of v3:
- DMA issue order reprioritized: all transfers serialize on the DMA device,
  so the first-needed bytes (wqk q-half, first 256 x cols) go first and the
  big rope tables are split/deferred; first matmul ~4us instead of ~14.5us
- rope tables in bf16; rope combine restructured: ACT does the psum->bf16
  copy, DVE does the two muls, gpsimd does the add (DVE was the 2nd-busiest
  engine); one rotate matmul per chunk covers q and k
- exp merged across the two heads ([128, 2, cols] strided AP, one scs PSUM
  tile spanning 2 banks); tril mask applied to both heads in one instr
- softmax denominators: reciprocal + stg mul read the PV PSUM directly
  (no pv_sb staging copy); stg for both heads lands in one tile/one DMA
- C-stage bias adds fused into one broadcast tensor_add per half
"""
import numpy as np

import concourse.bass as bass
import concourse.tile as tile
from concourse import bacc, mybir
from concourse.bass_utils import run_bass_kernel_spmd

B, L, D = 2, 2048, 1024
H, HD = 16, 64
BL = B * L                      # 4096
N_CORES = 8
H_PER = H // N_CORES            # 2
EV = H_PER * HD                 # 128
SEQ_SH = BL // N_CORES          # 512 output cols per core
OUT_B = SEQ_SH // B             # 256 per batch
F32 = mybir.dt.float32
F32R = mybir.dt.float32r
BF16 = mybir.dt.bfloat16
CHUNK = 256
N_CH_B = L // CHUNK             # 8 chunks per batch
KT = 128
N_KT_B = L // KT                # 16 kpos tiles per batch
QB = 512                        # attention q-block
MERGED_EXP = False
PV_COPY = False
ROPE_ADD_POOL = False
N_QB_B = L // QB                # 4 q-blocks per batch


def build(dup=1, no_cc=False, stage='full'):
    nc = bacc.Bacc("TRN2", target_bir_lowering=False, debug=False,
                   num_devices=N_CORES)
    xt = nc.dram_tensor("xt", [D, BL], BF16, kind="ExternalInput").ap()
    w_qk = nc.dram_tensor("w_qk", [D, 2 * EV], BF16, kind="ExternalInput").ap()
    w_v = nc.dram_tensor("w_v", [D, EV], BF16, kind="ExternalInput").ap()
    w_o = nc.dram_tensor("w_o", [D, D], BF16, kind="ExternalInput").ap()
    cos_pk = nc.dram_tensor("cos_pk", [EV, L], BF16, kind="ExternalInput").ap()
    sin_pk = nc.dram_tensor("sin_pk", [EV, L], BF16, kind="ExternalInput").ap()
    p2t = nc.dram_tensor("p2t", [EV, EV], BF16, kind="ExternalInput").ap()
    tril = nc.dram_tensor("tril", [KT, KT], BF16, kind="ExternalInput").ap()
    bias8 = nc.dram_tensor("bias8", [128, D // 128], F32,
                           kind="ExternalInput").ap()
    ones_in = nc.dram_tensor("ones_in", [65, 64], F32R,
                             kind="ExternalInput").ap()
    out = nc.dram_tensor("out", [D, SEQ_SH], F32, kind="ExternalOutput").ap()
    with tile.TileContext(nc) as tc:
        for it in range(dup):
            _emit(nc, tc, it, no_cc, stage, xt, w_qk, w_v, w_o, cos_pk,
                  sin_pk, p2t, tril, bias8, ones_in, out)
    nc.compile()
    return nc


def _emit(nc, tc, it, no_cc, stage, xt, w_qk, w_v, w_o, cos_pk, sin_pk,
          p2t, tril, bias8, ones_in, out):
    from contextlib import ExitStack
    s = f"_{it}"
    # one exchange buffer per (batch, q-block): core r ends up owning
    # output rows [j*512 + r*64, +64) of each batch; the C stage still works
    # on (batch, half-batch) groups of 128 q-cols
    HB = 128                     # q-cols per C-stage group
    bnc_in = {(b, j): nc.dram_tensor(f"bi{b}{j}{s}",
                                     [N_CORES * EV, 64], BF16)
              for b in range(B) for j in range(N_QB_B)}
    bnc_out = {(b, j): nc.dram_tensor(f"bo{b}{j}{s}",
                                      [N_CORES * EV, 64], BF16)
               for b in range(B) for j in range(N_QB_B)}
    xt_src = xt.rearrange("(c p) n -> p c n", p=128)
    wqk_src = w_qk.rearrange("(c p) n -> p c n", p=128)

    with ExitStack() as ctx:
        xqpool = ctx.enter_context(tc.tile_pool(name=f"xq{s}", bufs=2))
        cpool = ctx.enter_context(tc.tile_pool(name=f"c{s}", bufs=1))

        # every DMA serializes on the shared DMA device: issue in priority
        # order (first-needed bytes first), all on the sync queue
        wqk_all = cpool.tile([128, 8, 2 * EV], BF16, name=f"wqk{s}", tag="wqk")
        wv_all = cpool.tile([128, 8, EV], BF16, name=f"wv{s}", tag="wv")
        cos_sb = cpool.tile([EV, L], BF16, name=f"cos{s}", tag="cos")
        sin_sb = cpool.tile([EV, L], BF16, name=f"sin{s}", tag="sin")
        p2_sb = cpool.tile([128, 128], BF16, name=f"p2{s}", tag="p2")
        tril_sb = cpool.tile([KT, KT], BF16, name=f"tril{s}", tag="tril")
        bias_sb = cpool.tile([128, D // 128], F32, name=f"bias{s}", tag="bias")
        ones_sb = cpool.tile([65, 64], F32R, name=f"ones{s}", tag="ones")
        wo_all = cpool.tile([128, 8, D], BF16, name=f"wo{s}", tag="wo")
        xq0 = xqpool.tile([128, 8, 1024], BF16, tag="xq", name=f"xq0{s}")

        nc.sync.dma_start(wqk_all[:, 0:4, 0:EV], wqk_src[:, 0:4, 0:EV])
        nc.sync.dma_start(xq0[:, :, 0:256], xt_src[:, :, 0:256])
        nc.sync.dma_start(wqk_all[:, 4:8, 0:EV], wqk_src[:, 4:8, 0:EV])
        nc.sync.dma_start(wqk_all[:, :, EV:2 * EV], wqk_src[:, :, EV:2 * EV])
        nc.sync.dma_start(wv_all[:], w_v.rearrange("(c p) n -> p c n", p=128))
        nc.sync.dma_start(xq0[:, :, 256:512], xt_src[:, :, 256:512])
        nc.sync.dma_start(p2_sb[:], p2t[:])
        nc.sync.dma_start(tril_sb[:], tril[:])
        nc.sync.dma_start(ones_sb[:], ones_in[:])
        nc.sync.dma_start(cos_sb[:, 0:512], cos_pk[:, 0:512])
        nc.sync.dma_start(sin_sb[:, 0:512], sin_pk[:, 0:512])
        nc.sync.dma_start(xq0[:, :, 512:768], xt_src[:, :, 512:768])
        nc.sync.dma_start(xq0[:, :, 768:1024], xt_src[:, :, 768:1024])
        nc.sync.dma_start(cos_sb[:, 512:L], cos_pk[:, 512:L])
        nc.sync.dma_start(sin_sb[:, 512:L], sin_pk[:, 512:L])
        nc.sync.dma_start(bias_sb[:], bias8[:])

        # q in cols [0, BL), k in cols [BL, 2BL)
        qk2 = cpool.tile([128, 2 * BL], BF16, name=f"qk2{s}", tag="qk2")
        # 32 kpos tiles x [v_h0(64) | ones | v_h1(64) | ones]
        va_all = cpool.tile([128, 2 * N_KT_B * 130], BF16, name=f"va{s}",
                            tag="va")
        nc.vector.memset(
            va_all[:].rearrange("p (t w) -> p t w", w=65)[:, :, 64:65], 1.0)

        def load_xq(qtr):
            t = xqpool.tile([128, 8, 1024], BF16, tag="xq",
                            name=f"xq{qtr}{s}")
            nc.sync.dma_start(t[:, :, 0:512],
                              xt_src[:, :, qtr * 1024:qtr * 1024 + 512])
            nc.sync.dma_start(t[:, :, 512:1024],
                              xt_src[:, :, qtr * 1024 + 512:(qtr + 1) * 1024])
            return t

        with ExitStack() as abctx:
            apool = abctx.enter_context(tc.tile_pool(name=f"pa{s}", bufs=2))
            epool = abctx.enter_context(tc.tile_pool(name=f"pex{s}", bufs=4))
            sps = abctx.enter_context(
                tc.tile_pool(name=f"psps{s}", bufs=1, space="PSUM"))
            pps = abctx.enter_context(
                tc.tile_pool(name=f"ppps{s}", bufs=1, space="PSUM"))
            bps = abctx.enter_context(
                tc.tile_pool(name=f"pbps{s}", bufs=1, space="PSUM"))
            npool = abctx.enter_context(tc.tile_pool(name=f"pn{s}", bufs=2))
            # opened last so they can be released mid-stream (stack order)
            apsx = ExitStack()
            abctx.enter_context(apsx)
            aps = apsx.enter_context(
                tc.tile_pool(name=f"paps{s}", bufs=1, space="PSUM"))
            rps = apsx.enter_context(
                tc.tile_pool(name=f"prps{s}", bufs=1, space="PSUM"))
            vps = apsx.enter_context(
                tc.tile_pool(name=f"pvps{s}", bufs=1, space="PSUM"))

            from collections import deque
            xq_tiles = {0: xq0}
            units = deque()

            def push_chunk_units(c):
                # c: global 256-col chunk index in [0, 16)
                st = {}

                def u_qk(e):
                    qtr, off = c // 4, (c % 4) * CHUNK
                    if e == 0:
                        if c % 4 == 0 and qtr not in xq_tiles:
                            xq_tiles[qtr] = load_xq(qtr)
                        if c % 4 == 2 and qtr + 1 <= 3:
                            xq_tiles[qtr + 1] = load_xq(qtr + 1)  # prefetch
                        st["x"] = xq_tiles[qtr]
                        st["ps"] = aps.tile([128, 2, CHUNK], F32, tag="qk",
                                            name=f"qk{c}{s}")
                    for cc in range(8):
                        nc.tensor.matmul(
                            st["ps"][:, e, :],
                            wqk_all[:, cc, e * EV:(e + 1) * EV],
                            st["x"][:, cc, off:off + CHUNK],
                            start=(cc == 0), stop=(cc == 7))

                def u_vt(t_):
                    off = (c % 4) * CHUNK
                    if t_ == 0:
                        st["pv"] = vps.tile([128, 2 * KT], F32, tag="vt",
                                            name=f"vt{c}{s}")
                    for cc in range(8):
                        nc.tensor.matmul(
                            st["pv"][:, t_ * KT:(t_ + 1) * KT],
                            st["x"][:, cc, off + t_ * KT:off + (t_ + 1) * KT],
                            wv_all[:, cc, :],
                            start=(cc == 0), stop=(cc == 7))
                    if t_ == 1:
                        nc.vector.tensor_copy(
                            va_all[:, c * 260:(c + 1) * 260]
                            .rearrange("p (g w) -> p g w", w=65)[:, :, 0:64],
                            st["pv"][:].rearrange("p (g w) -> p g w", w=64))

                def u_rope():
                    b = c // N_CH_B
                    lcol = c * CHUNK - b * L
                    ps = st["ps"]
                    sb = apool.tile([128, 2, CHUNK], BF16, tag="sb",
                                    name=f"sb{c}{s}")
                    nc.scalar.activation(sb[:], ps[:],
                                         mybir.ActivationFunctionType.Copy)
                    pr = rps.tile([128, 2, CHUNK], F32, tag="rot",
                                  name=f"rot{c}{s}")
                    nc.tensor.matmul(
                        pr[:].rearrange("p g n -> p (g n)"), p2_sb[:],
                        sb[:].rearrange("p g n -> p (g n)"),
                        start=True, stop=True)
                    cos_bc = (cos_sb[:, lcol:lcol + CHUNK].unsqueeze(1)
                              .broadcast_to([EV, 2, CHUNK]))
                    sin_bc = (sin_sb[:, lcol:lcol + CHUNK].unsqueeze(1)
                              .broadcast_to([EV, 2, CHUNK]))
                    t1 = apool.tile([128, 2, CHUNK], BF16, tag="t1",
                                    name=f"t1{c}{s}")
                    nc.vector.tensor_mul(t1[:], sb[:], cos_bc)
                    t2 = apool.tile([128, 2, CHUNK], F32, tag="t2",
                                    name=f"t2{c}{s}")
                    nc.vector.tensor_mul(t2[:], pr[:], sin_bc)
                    dst = (qk2[:].rearrange("p (g n) -> p g n", g=2)
                           [:, :, c * CHUNK:(c + 1) * CHUNK])
                    eng = nc.gpsimd if ROPE_ADD_POOL else nc.vector
                    eng.tensor_add(dst, t1[:], t2[:])

                units.append(lambda: u_qk(0))
                units.append(lambda: u_qk(1))
                units.append(u_rope)
                units.append(lambda: u_vt(0))
                units.append(lambda: u_vt(1))

            def emit_a_chunk(c):
                push_chunk_units(c)
                while units:
                    units.popleft()()

            def emit_pv(b, j, ki, n_ki, exs, pvs):
                qlo = max(0, ki * KT - j * QB)
                for h in range(2):
                    nc.tensor.matmul(
                        pvs[h][:, qlo:QB],
                        va_all[:, (b * N_KT_B + ki) * 130 + h * 65:
                               (b * N_KT_B + ki) * 130 + (h + 1) * 65],
                        exs[h][:, qlo:QB],
                        start=(ki == 0), stop=(ki == n_ki - 1))

            def emit_b_qb(b, j):
                q0 = b * L + j * QB
                n_ki = 4 * (j + 1)
                pvs = [pps.tile([65, QB], F32, tag=f"pv{h}",
                                name=f"pv{h}{b}{j}{s}") for h in range(2)]
                prev = None
                for ki in range(n_ki):
                    qlo = max(0, ki * KT - j * QB)
                    kcols = slice(BL + b * L + ki * KT,
                                  BL + b * L + (ki + 1) * KT)
                    if MERGED_EXP:
                        sc = sps.tile([128, 2, QB], F32, tag="sc",
                                      name=f"sc{b}{j}{ki}{s}")
                        for h in range(2):
                            nc.tensor.matmul(
                                sc[:, h, qlo:QB],
                                qk2[h * 64:(h + 1) * 64, kcols],
                                qk2[h * 64:(h + 1) * 64,
                                    q0 + qlo:q0 + QB],
                                start=True, stop=True)
                        exb = epool.tile([128, 2, QB], BF16, tag="ex",
                                         name=f"ex{b}{j}{ki}{s}")
                        nc.scalar.activation(
                            exb[:, :, qlo:QB], sc[:, :, qlo:QB],
                            mybir.ActivationFunctionType.Exp)
                        if ki >= 4 * j:
                            nc.vector.tensor_mul(
                                exb[:, :, qlo:qlo + KT],
                                exb[:, :, qlo:qlo + KT],
                                tril_sb[:].unsqueeze(1)
                                .broadcast_to([KT, 2, KT]))
                        exs = [exb[:, 0], exb[:, 1]]
                    else:
                        scs, exs = [], []
                        for h in range(2):
                            sc = sps.tile([128, QB], F32, tag=f"sc{h}",
                                          name=f"sc{h}{b}{j}{ki}{s}")
                            nc.tensor.matmul(
                                sc[:, qlo:QB],
                                qk2[h * 64:(h + 1) * 64, kcols],
                                qk2[h * 64:(h + 1) * 64,
                                    q0 + qlo:q0 + QB],
                                start=True, stop=True)
                            scs.append(sc)
                        for h in range(2):
                            ex = epool.tile([128, QB], BF16, tag=f"ex{h}",
                                            name=f"ex{h}{b}{j}{ki}{s}")
                            nc.scalar.activation(
                                ex[:, qlo:QB], scs[h][:, qlo:QB],
                                mybir.ActivationFunctionType.Exp)
                            if ki >= 4 * j:
                                nc.vector.tensor_mul(
                                    ex[:, qlo:qlo + KT], ex[:, qlo:qlo + KT],
                                    tril_sb[:])
                            exs.append(ex)
                    if prev is not None:
                        emit_pv(b, j, prev[0], n_ki, prev[1], pvs)
                    prev = (ki, exs)
                    # interleave pipelined filler work into the exp shadow
                    rem = n_ki - ki
                    k = max(1, len(units) // rem) if units else 0
                    for _ in range(k):
                        if units:
                            units.popleft()()
                emit_pv(b, j, prev[0], n_ki, prev[1], pvs)
                stg = npool.tile([128, QB], BF16, tag="stg",
                                 name=f"stg{b}{j}{s}")
                rec = npool.tile([65, QB], F32R, tag="rec",
                                 name=f"rec{b}{j}{s}")
                for h in range(2):
                    pv_sb = npool.tile([65, QB], F32, tag=f"pvsb{h}",
                                       name=f"pvsb{h}{b}{j}{s}")
                    nc.vector.tensor_copy(pv_sb[:], pvs[h][:])
                    with nc.allow_low_precision(reason="recip for bcast"):
                        nc.vector.reciprocal(rec[64:65, :], pv_sb[64:65, :])
                    bc = bps.tile([64, QB], F32, tag="bc",
                                  name=f"bc{h}{b}{j}{s}")
                    nc.tensor.matmul(bc[:], ones_sb[64:65, 0:64],
                                     rec[64:65, :], start=True, stop=True)
                    nc.vector.tensor_mul(stg[h * 64:(h + 1) * 64, :],
                                         pv_sb[0:64, :], bc[:])
                nc.sync.dma_start(
                    bnc_in[(b, j)].rearrange("(dc r) n -> r dc n", r=EV),
                    stg[:].rearrange("p (g n) -> p g n", g=8))

            def emit_a2a(b, j):
                if no_cc:
                    nc.sync.dma_start(bnc_out[(b, j)][:], bnc_in[(b, j)][:])
                else:
                    nc.gpsimd.collective_compute(
                        "AllToAll", mybir.AluOpType.bypass,
                        replica_groups=[list(range(N_CORES))],
                        ins=[bnc_in[(b, j)][:].opt()],
                        outs=[bnc_out[(b, j)][:].opt()])

            cst = {}

            def push_rb_half(bb, j):
                hb, sub = j // 2, j % 2

                def u_rbh():
                    key = f"rb{bb}{hb}"
                    if key not in cst:
                        cst[key] = cpool.tile([128, 8, HB], BF16, tag=key,
                                              name=f"{key}{s}")
                    nc.sync.dma_start(
                        cst[key][:, :, sub * 64:(sub + 1) * 64],
                        bnc_out[(bb, j)].rearrange("(c p) n -> p c n",
                                                   p=128))
                units.append(u_rbh)

            def push_c_units(bb, hb, cps):
                def u_po(half, dd4):
                    dd = half * 4 + dd4
                    if dd4 == 0:
                        cst[(bb, hb, half)] = cps.tile(
                            [128, 4, HB], F32, tag=f"po{half}",
                            name=f"po{bb}{hb}{half}{s}")
                    for cc in range(8):
                        nc.tensor.matmul(
                            cst[(bb, hb, half)][:, dd4, :],
                            wo_all[:, cc, dd * 128:(dd + 1) * 128],
                            cst[f"rb{bb}{hb}"][:, cc, :],
                            start=(cc == 0), stop=(cc == 7))

                def u_fo(half):
                    fo = cpool.tile([128, 4, HB], F32,
                                    tag=f"fo{bb}{hb}{half}",
                                    name=f"fo{bb}{hb}{half}{s}")
                    bias_bc = (bias_sb[:, half * 4:(half + 1) * 4]
                               .unsqueeze(2).broadcast_to([128, 4, HB]))
                    nc.vector.tensor_add(fo[:], cst[(bb, hb, half)][:],
                                         bias_bc)
                    nc.sync.dma_start(
                        out.rearrange("(dd p) n -> p dd n", p=128)
                        [:, half * 4:(half + 1) * 4,
                         bb * OUT_B + hb * HB:bb * OUT_B + (hb + 1) * HB],
                        fo[:])

                for half in range(2):
                    for dd4 in range(4):
                        units.append(
                            (lambda h_, d_: lambda: u_po(h_, d_))(half, dd4))
                    units.append((lambda h_: lambda: u_fo(h_))(half))

            if stage == 'A':
                for c in range(2 * N_CH_B):
                    emit_a_chunk(c)
            else:
                emit_a_chunk(0)
                emit_a_chunk(1)
                cps = None
                for b in range(B):
                    for j in range(N_QB_B):
                        cp = b * N_QB_B + j + 1
                        if cp <= 7:
                            push_chunk_units(2 * cp)
                            push_chunk_units(2 * cp + 1)
                        if stage == 'full':
                            # rb loads as soon as the matching A2A landed
                            if b == 1 and j == 0:
                                push_rb_half(0, 0)
                                push_rb_half(0, 1)
                            if b == 1 and j == 1:
                                push_rb_half(0, 2)
                                push_rb_half(0, 3)
                            if b == 1 and j == 2:
                                push_rb_half(1, 0)
                                push_rb_half(1, 1)
                            if b == 1 and j == 3:
                                # projection psums done: free banks for C
                                apsx.close()
                                cps = abctx.enter_context(tc.tile_pool(
                                    name=f"pcps{s}", bufs=1, space="PSUM"))
                                push_c_units(0, 0, cps)
                                push_c_units(0, 1, cps)
                                push_c_units(1, 0, cps)
                                push_rb_half(1, 2)
                        emit_b_qb(b, j)
                        emit_a2a(b, j)
                    while units:
                        units.popleft()()
                    if b == 0:
                        nc.sync.dma_start(
                            wo_all[:, :, 0:512],
                            w_o.rearrange("(c p) n -> p c n", p=128)
                            [:, :, 0:512])
                        nc.sync.dma_start(
                            wo_all[:, :, 512:1024],
                            w_o.rearrange("(c p) n -> p c n", p=128)
                            [:, :, 512:1024])

        if stage == 'A':
            nc.sync.dma_start(out[0:128, 0:256],
                              qk2[:, 0:512].bitcast(F32))
            return
        if stage == 'AB':
            nc.sync.dma_start(out[0:128, 0:64],
                              bnc_out[(0, 0)][0:128, :].bitcast(F32))
            return

        # tail: C for (batch 1, half 1)
        with ExitStack() as cctx:
            cps2 = cctx.enter_context(
                tc.tile_pool(name=f"pcps2{s}", bufs=1, space="PSUM"))
            push_rb_half(1, 3)
            push_c_units(1, 1, cps2)
            while units:
                units.popleft()()


def _host_prep(x, rope_cos, rope_sin, W_qkv, W_out, b_out):
    import ml_dtypes
    bf = ml_dtypes.bfloat16
    x = np.asarray(x, np.float32)
    rope_cos = np.asarray(rope_cos, np.float32)
    rope_sin = np.asarray(rope_sin, np.float32)
    W_qkv = np.asarray(W_qkv, np.float32)
    W_out = np.asarray(W_out, np.float32)
    b_out = np.asarray(b_out, np.float32)

    xt = np.ascontiguousarray(x.reshape(BL, D).T).astype(bf)
    cos_pk = np.ascontiguousarray(np.tile(rope_cos[:L].T, (2, 1))).astype(bf)
    sin_pk = np.ascontiguousarray(np.tile(rope_sin[:L].T, (2, 1))).astype(bf)
    P = np.zeros((HD, HD), np.float32)
    for i in range(32):
        P[i, i + 32] = -1.0
        P[i + 32, i] = 1.0
    P2 = np.zeros((EV, EV), np.float32)
    P2[:HD, :HD] = P
    P2[HD:, HD:] = P
    p2t = np.ascontiguousarray(P2.T).astype(bf)
    tril_m = np.triu(np.ones((KT, KT), np.float32)).astype(bf)
    bias8 = np.ascontiguousarray(b_out.reshape(D // 128, 128).T)
    w_o = W_out.astype(bf)
    ones65 = np.ones((65, 64), np.float32)
    scale = HD ** -0.5

    in_maps = []
    for r in range(N_CORES):
        hs = slice(r * EV, (r + 1) * EV)
        w_qk = np.concatenate(
            [W_qkv[:, 0:1024][:, hs] * scale, W_qkv[:, 1024:2048][:, hs]],
            axis=1).astype(bf)
        w_v = W_qkv[:, 2048:3072][:, hs].astype(bf)
        in_maps.append({
            "xt": xt, "w_qk": w_qk, "w_v": w_v, "w_o": w_o,
            "cos_pk": cos_pk, "sin_pk": sin_pk, "p2t": p2t,
            "tril": tril_m, "bias8": bias8, "ones_in": ones65,
        })
    return in_maps


_NC_CACHE = {}


def kernel(x, rope_cos, rope_sin, W_qkv, W_out, b_out):
    if "nc" not in _NC_CACHE:
        _NC_CACHE["nc"] = build()
    nc = _NC_CACHE["nc"]
    in_maps = _host_prep(x, rope_cos, rope_sin, W_qkv, W_out, b_out)
    res = run_bass_kernel_spmd(nc, in_maps, core_ids=list(range(N_CORES)))
    outp = np.empty((BL, D), np.float32)
    for r in range(N_CORES):
        o = res.results[r]["out"]          # [D, 512]
        for b in range(B):
            for j in range(N_QB_B):
                row0 = b * L + j * 512 + r * 64
                col0 = b * OUT_B + (j // 2) * 128 + (j % 2) * 64
                outp[row0:row0 + 64, :] = o[:, col0:col0 + 64].T
    return outp.reshape(B, L, D)
